# revision 6
# baseline (speedup 1.0000x reference)
"""Trainium2 Bass kernel for nn_CLIP_26869315404321 (contrastive loss_fn).

Computation (B=4096, D=2048, P=512):
    Q = LN(sim1 @ Wq + bq); S = LN(sim2 @ Wk + bk)
    sim = Q @ S.T - 1000*(eye + same_uid)   (eye is subsumed by same_uid)
    sim_sm = softmax(sim, axis=1)
    recon = sim_sm @ input_for_reconstruction
    returns (recon, sim_sm)

Sharding: batch dim B split across 8 cores (512 rows each). Each core
computes its own Q and S row-shard, the S shards are exchanged with an
on-device AllGather (packed fp16 hi/lo), and each core finishes its row
block of sim/softmax/recon. Host code slices inputs per core and
concatenates the row-shard outputs.

Precision: the projection and similarity matmuls use an fp16 hi/lo
3-matmul split (~1e-7 relative, needed because softmax amplifies absolute
logit error by sqrt(K)); the reconstruction matmul uses float32r.
"""

import os

import numpy as np

import bass_rust
import concourse.bass as bass
import concourse.mybir as mybir
import concourse.tile as tile_mod
from concourse.bass_utils import run_bass_kernel_spmd
from concourse.masks import make_identity
from concourse.tile import TileContext

# ---------------------------------------------------------------------------
# Compat: this container's walrus build rejects instructions carrying more
# than one sync-wait command. (1) Replace Tile's exit drain (which collects
# one wait per logical proc) with per-proc SP nops. (2) After tracing, hoist
# excess waits from any instruction onto same-engine NoOps placed just
# before it -- engine queues execute in order, so semantics are unchanged.
# ---------------------------------------------------------------------------


def _patched_drain(self, tick_clock, wait_clock):
    gc = tick_clock.global_clock
    vals = eval(repr(gc).replace("VectorClock", ""))
    n = len(vals)
    for proc in range(n):
        if vals[proc] > 0:
            single = [0] * n
            single[proc] = vals[proc]
            nop = self.nc.sync.nop(nofuse=True, hint="split_drain_wait")
            wait_clock.add_sem_waits(
                nop.ins, bass_rust.ScopedClock({None: bass_rust.VectorClock(single)})
            )
    self.nc.sync.drain()
    self.nc.all_engine_barrier()
    assert self.sems is not None
    popped = self.nc._tile_sem_poison_stack.pop()
    assert popped is self._sem_poison
    self.nc.clear_and_free_semaphores(list(self.sems.allocated().values()))
    self.nc.all_engine_barrier()


tile_mod.TileContext._drain_and_barrier = _patched_drain


def _split_multi_waits(nc, max_waits=1):
    n_split = 0
    for f in nc.m.functions:
        for blk in f.blocks:
            if not any(
                ins.sync_info is not None
                and ins.sync_info.on_wait
                and len(ins.sync_info.on_wait) > max_waits
                for ins in blk.instructions
            ):
                continue
            newlist = []
            for ins in blk.instructions:
                si = ins.sync_info
                waits = list(si.on_wait) if (si is not None and si.on_wait) else []
                if len(waits) > max_waits:
                    extra = waits[:-max_waits]
                    keep = waits[-max_waits:]
                    for i, w in enumerate(extra):
                        n_split += 1
                        newlist.append(
                            mybir.InstNoOp(
                                name=f"{ins.name}-hw{i}",
                                engine=ins.engine,
                                ins=[],
                                outs=[],
                                sync_info=mybir.SyncInfo(on_wait=[w], on_update=[]),
                            )
                        )
                    ins.sync_info = mybir.SyncInfo(
                        on_wait=keep, on_update=list(si.on_update or [])
                    )
                newlist.append(ins)
            blk.instructions[:] = newlist
    return n_split


# ---------------------------------------------------------------------------

B, D, P = 4096, 2048, 512
NCORES = 8
BL = B // NCORES  # 512 rows per core
RC = BL // 128  # 4 row chunks
KC = D // 128  # 16 contraction chunks for the projections
FC = P // 128  # 4 feature chunks
NS = B // 512  # 8 similarity column blocks
NJ = B // 128  # 32 contraction chunks for recon
LN_EPS = 1e-5
MASK_VAL = 1000.0

f32 = mybir.dt.float32
f32r = mybir.dt.float32r
f16 = mybir.dt.float16
i32 = mybir.dt.int32
ALU = mybir.AluOpType
ACTF = mybir.ActivationFunctionType
AX = mybir.AxisListType


def _build_module():
    from contextlib import ExitStack

    nc = bass.Bass(num_devices=NCORES)

    xq = nc.dram_tensor("xq", [BL, D], f32, kind="ExternalInput")
    xk = nc.dram_tensor("xk", [BL, D], f32, kind="ExternalInput")
    inp = nc.dram_tensor("inp", [B, D], f32, kind="ExternalInput")
    wq_d = nc.dram_tensor("wq", [D, P], f32, kind="ExternalInput")
    wk_d = nc.dram_tensor("wk", [D, P], f32, kind="ExternalInput")
    bq_d = nc.dram_tensor("bq", [P], f32, kind="ExternalInput")
    bk_d = nc.dram_tensor("bk", [P], f32, kind="ExternalInput")
    g_d = nc.dram_tensor("g", [P], f32, kind="ExternalInput")
    b_d = nc.dram_tensor("b", [P], f32, kind="ExternalInput")
    u_all_d = nc.dram_tensor("u_all", [B], i32, kind="ExternalInput")
    u_loc_d = nc.dram_tensor("u_loc", [BL], i32, kind="ExternalInput")

    recon_d = nc.dram_tensor("recon", [BL, D], f32, kind="ExternalOutput")
    sim_sm_d = nc.dram_tensor("sim_sm", [BL, B], f32, kind="ExternalOutput")

    # AllGather payload: fp16 hi rows [0:P], fp16 lo rows [P:2P], both
    # feature-major [feat, local_row].
    cc_in = nc.dram_tensor("cc_in", [2 * P, BL], f16, kind="Internal")
    cc_out = nc.dram_tensor(
        "cc_out", [NCORES * 2 * P, BL], f16, kind="Internal", addr_space="Shared"
    )

    with TileContext(nc) as tc, ExitStack() as ctx:
        const = ctx.enter_context(tc.tile_pool(name="const", bufs=1))
        small = ctx.enter_context(tc.tile_pool(name="small", bufs=8))

        ident = const.tile([128, 128], f32)
        make_identity(nc, ident[:])
        eps_t = const.tile([128, 1], f32)
        nc.gpsimd.memset(eps_t[:], LN_EPS)

        gpf = const.tile([128, FC], f32)
        nc.sync.dma_start(gpf[:], g_d.rearrange("(fc p) -> p fc", p=128))
        bpf = const.tile([128, FC], f32)
        nc.sync.dma_start(bpf[:], b_d.rearrange("(fc p) -> p fc", p=128))
        bq_b = const.tile([128, P], f32)
        nc.sync.dma_start(bq_b[:], bq_d[:].partition_broadcast(128))
        bk_b = const.tile([128, P], f32)
        nc.sync.dma_start(bk_b[:], bk_d[:].partition_broadcast(128))

        u_all_f = const.tile([128, B], f32)
        u_loc_f = const.tile([128, RC], f32)
        with tc.tile_pool(name="utmp", bufs=1) as utmp:
            u_all_i = utmp.tile([128, B], i32)
            nc.sync.dma_start(u_all_i[:], u_all_d[:].partition_broadcast(128))
            nc.vector.tensor_copy(u_all_f[:], u_all_i[:])
            u_loc_i = utmp.tile([128, RC], i32)
            nc.sync.dma_start(u_loc_i[:], u_loc_d.rearrange("(rc p) -> p rc", p=128))
            nc.vector.tensor_copy(u_loc_f[:], u_loc_i[:])

        # simT lives until the recon matmuls at the very end, so its pool is
        # opened first (pool frees must be LIFO).
        stt = ctx.enter_context(tc.tile_pool(name="stt", bufs=1))
        simT = stt.tile([128, NJ, BL], f32r, tag="simT")

        qsp = ctx.enter_context(tc.tile_pool(name="qsp", bufs=1))
        s_h = qsp.tile([128, FC, BL], f16, tag="sh")
        s_l = qsp.tile([128, FC, BL], f16, tag="sl")
        q_h = qsp.tile([128, FC, BL], f16, tag="qh")
        q_l = qsp.tile([128, FC, BL], f16, tag="ql")

        with (
            tc.tile_pool(name="wpool", bufs=2) as wpool,
            tc.tile_pool(name="xpool", bufs=1) as xpool,
            tc.tile_pool(name="xtp", bufs=3) as xtp,
            tc.tile_pool(name="prjp", bufs=2) as prjp,
            tc.tile_pool(name="pt_ps", bufs=3, space="PSUM") as pt_ps,
            tc.tile_pool(name="pp_ps", bufs=1, space="PSUM") as pp_ps,
        ):

            def proj_ln_t(x_d, w_d, bias_b, out_h, out_l, side):
                """out_h/out_l: [128, FC, BL] fp16 hi/lo of (LN(x@W+b)*g+b).T."""
                xnat = xpool.tile([128, RC, D], f32, tag="xnat")
                for rc in range(RC):
                    nc.sync.dma_start(
                        xnat[:, rc, :], x_d[rc * 128 : (rc + 1) * 128, :]
                    )
                psum_rc = [
                    pp_ps.tile([128, P], f32, tag=f"pp{rc}", name=f"pp{rc}_{side}")
                    for rc in range(RC)
                ]
                for kc in range(KC):
                    w_f = wpool.tile([128, P], f32, tag="w32")
                    nc.sync.dma_start(w_f[:], w_d[kc * 128 : (kc + 1) * 128, :])
                    w_h = wpool.tile([128, P], f16, tag="wh")
                    nc.scalar.copy(w_h[:], w_f[:])
                    w_l = wpool.tile([128, P], f16, tag="wl")
                    nc.vector.tensor_tensor(w_l[:], w_f[:], w_h[:], ALU.subtract)
                    xt_h = xtp.tile([128, RC, 128], f16, tag="xth")
                    xt_l = xtp.tile([128, RC, 128], f16, tag="xtl")
                    for rc in range(RC):
                        pst = pt_ps.tile([128, 128], f32, tag="pt")
                        nc.tensor.transpose(
                            pst[:], xnat[:, rc, kc * 128 : (kc + 1) * 128], ident[:]
                        )
                        nc.scalar.copy(xt_h[:, rc, :], pst[:])
                        nc.vector.tensor_tensor(
                            xt_l[:, rc, :], pst[:], xt_h[:, rc, :], ALU.subtract
                        )
                    for rc in range(RC):
                        for i, (lh, rh) in enumerate(
                            (
                                (xt_h[:, rc, :], w_h[:]),
                                (xt_h[:, rc, :], w_l[:]),
                                (xt_l[:, rc, :], w_h[:]),
                            )
                        ):
                            nc.tensor.matmul(
                                psum_rc[rc][:],
                                lh,
                                rh,
                                start=(kc == 0 and i == 0),
                                stop=(kc == KC - 1 and i == 2),
                            )

                for rc in range(RC):
                    prj = prjp.tile([128, P], f32, tag="prj")
                    nc.vector.tensor_tensor(
                        prj[:], psum_rc[rc][:], bias_b[:], ALU.add
                    )
                    st6 = small.tile([128, 6], f32, tag="st6")
                    nc.vector.bn_stats(st6[:], prj[:])
                    mv = small.tile([128, 2], f32, tag="mv")
                    nc.vector.bn_aggr(mv[:], st6[:])
                    std = small.tile([128, 1], f32, tag="s1")
                    nc.scalar.activation(std[:], mv[:, 1:2], ACTF.Sqrt, bias=eps_t[:])
                    rstd = small.tile([128, 1], f32, tag="s1")
                    nc.vector.reciprocal(rstd[:], std[:])
                    nmr = small.tile([128, 1], f32, tag="s1")
                    nc.vector.tensor_tensor(nmr[:], mv[:, 0:1], rstd[:], ALU.mult)
                    nc.vector.tensor_scalar(nmr[:], nmr[:], -1.0, None, ALU.mult)
                    tn = prjp.tile([128, P], f32, tag="tn")
                    nc.vector.tensor_scalar(
                        tn[:], prj[:], rstd[:], nmr[:], ALU.mult, ALU.add
                    )
                    # transpose back to feature-major, apply g/b, split hi/lo
                    for fc in range(FC):
                        pst = pt_ps.tile([128, 128], f32, tag="pt")
                        nc.tensor.transpose(
                            pst[:], tn[:, fc * 128 : (fc + 1) * 128], ident[:]
                        )
                        qf = prjp.tile([128, 128], f32, tag="qf")
                        nc.vector.tensor_scalar(
                            qf[:],
                            pst[:],
                            gpf[:, fc : fc + 1],
                            bpf[:, fc : fc + 1],
                            ALU.mult,
                            ALU.add,
                        )
                        hs = out_h[:, fc, rc * 128 : (rc + 1) * 128]
                        nc.scalar.copy(hs, qf[:])
                        nc.vector.tensor_tensor(
                            out_l[:, fc, rc * 128 : (rc + 1) * 128],
                            qf[:],
                            hs,
                            ALU.subtract,
                        )

            # ---------------- supports shard -> AllGather ----------------
            proj_ln_t(xk, wk_d, bk_b, s_h, s_l, "s")
            nc.sync.dma_start(
                cc_in[0:P, :].rearrange("(fc p) r -> p fc r", p=128), s_h[:]
            )
            nc.sync.dma_start(
                cc_in[P : 2 * P, :].rearrange("(fc p) r -> p fc r", p=128), s_l[:]
            )
            nc.gpsimd.collective_compute(
                "AllGather",
                ALU.bypass,
                ins=[cc_in[:]],
                outs=[cc_out[:]],
                replica_groups=[list(range(NCORES))],
            )

            # ------------- queries shard (overlaps the AllGather) -------------
            proj_ln_t(xq, wq_d, bq_b, q_h, q_l, "q")

        # ---------------- similarity + mask (ST streamed per column block) ----
        simp_es = ExitStack()
        simp = simp_es.enter_context(tc.tile_pool(name="simp", bufs=1))
        sim_sb = simp.tile([128, RC, B], f32, tag="sim")

        with (
            tc.tile_pool(name="stq", bufs=3) as stq,
            tc.tile_pool(name="mskp", bufs=2) as mskp,
            tc.tile_pool(name="sim_ps", bufs=3, space="PSUM") as sim_ps,
        ):
            for ns in range(NS):
                r = ns  # one gathered rank block per 512-column block
                base = r * 2 * P
                stb_h = stq.tile([128, FC, 512], f16, tag="sth")
                nc.sync.dma_start(
                    stb_h[:],
                    cc_out[base : base + P, :].rearrange("(fc p) j -> p fc j", p=128),
                )
                stb_l = stq.tile([128, FC, 512], f16, tag="stl")
                nc.sync.dma_start(
                    stb_l[:],
                    cc_out[base + P : base + 2 * P, :].rearrange(
                        "(fc p) j -> p fc j", p=128
                    ),
                )
                for mc in range(RC):
                    ps = sim_ps.tile([128, 512], f32, tag="ps")
                    i_mm = 0
                    n_mm = 3 * FC
                    for fc in range(FC):
                        for lh, rh in (
                            (q_h[:, fc, mc * 128 : (mc + 1) * 128], stb_h[:, fc, :]),
                            (q_h[:, fc, mc * 128 : (mc + 1) * 128], stb_l[:, fc, :]),
                            (q_l[:, fc, mc * 128 : (mc + 1) * 128], stb_h[:, fc, :]),
                        ):
                            nc.tensor.matmul(
                                ps[:],
                                lh,
                                rh,
                                start=(i_mm == 0),
                                stop=(i_mm == n_mm - 1),
                            )
                            i_mm += 1
                    msk = mskp.tile([128, 512], f32, tag="msk")
                    nc.vector.tensor_scalar(
                        msk[:],
                        u_all_f[:, ns * 512 : (ns + 1) * 512],
                        u_loc_f[:, mc : mc + 1],
                        -MASK_VAL,
                        ALU.is_equal,
                        ALU.mult,
                    )
                    nc.vector.tensor_tensor(
                        sim_sb[:, mc, ns * 512 : (ns + 1) * 512], ps[:], msk[:], ALU.add
                    )

        # ---------------- softmax (rows) + write sim_sm ----------------
        for mc in range(RC):
            row = sim_sb[:, mc, :]
            nmx = small.tile([128, 1], f32, tag="s1")
            nc.vector.tensor_reduce(nmx[:], row, axis=AX.X, op=ALU.max, negate=True)
            ssum = small.tile([128, 1], f32, tag="s1")
            nc.scalar.activation(
                row, row, ACTF.Exp, bias=nmx[:], scale=1.0, accum_out=ssum[:]
            )
            rs = small.tile([128, 1], f32, tag="s1")
            nc.vector.reciprocal(rs[:], ssum[:])
            nc.vector.tensor_scalar(row, row, rs[:], None, ALU.mult)
            nc.scalar.dma_start(sim_sm_d[mc * 128 : (mc + 1) * 128, :], row)

        # ---------------- transpose sim_sm for recon ----------------
        with tc.tile_pool(name="pt2_ps", bufs=3, space="PSUM") as pt2_ps:
            for mc in range(RC):
                for j in range(NJ):
                    pst = pt2_ps.tile([128, 128], f32, tag="pt2")
                    nc.tensor.transpose(
                        pst[:], sim_sb[:, mc, j * 128 : (j + 1) * 128], ident[:]
                    )
                    nc.vector.tensor_copy(simT[:, j, mc * 128 : (mc + 1) * 128], pst[:])
        simp_es.close()

        # ---------------- recon = sim_sm @ inp ----------------
        inpp = ctx.enter_context(tc.tile_pool(name="inpp", bufs=4))
        recp = ctx.enter_context(tc.tile_pool(name="recp", bufs=4))
        rec_ps = ctx.enter_context(tc.tile_pool(name="rec_ps", bufs=1, space="PSUM"))
        DG = 2  # halves of D; 8 psum banks = 4 row-chunks x 2 columns of 512
        for dg in range(DG):
            rps = [
                [
                    rec_ps.tile(
                        [128, 512], f32, tag=f"pr_{mc}_{dc}", name=f"pr_{mc}_{dc}"
                    )
                    for dc in range(2)
                ]
                for mc in range(RC)
            ]
            for j in range(NJ):
                it = inpp.tile([128, 1024], f32r, tag="it")
                nc.sync.dma_start(
                    it[:],
                    inp[j * 128 : (j + 1) * 128, dg * 1024 : (dg + 1) * 1024].bitcast(
                        f32r
                    ),
                )
                for mc in range(RC):
                    for dc in range(2):
                        nc.tensor.matmul(
                            rps[mc][dc][:],
                            simT[:, j, mc * 128 : (mc + 1) * 128],
                            it[:, dc * 512 : (dc + 1) * 512],
                            start=(j == 0),
                            stop=(j == NJ - 1),
                        )
            for mc in range(RC):
                for dc in range(2):
                    rec = recp.tile([128, 512], f32, tag="rec")
                    nc.vector.tensor_copy(rec[:], rps[mc][dc][:])
                    nc.scalar.dma_start(
                        recon_d[
                            mc * 128 : (mc + 1) * 128,
                            dg * 1024 + dc * 512 : dg * 1024 + (dc + 1) * 512,
                        ],
                        rec[:],
                    )

    _split_multi_waits(nc)
    return nc


_CACHE = {}
LAST_RESULTS = None


def _get_nc():
    if "nc" not in _CACHE:
        _CACHE["nc"] = _build_module()
    return _CACHE["nc"]


def kernel(
    sim1,
    sim2,
    input_for_reconstruction,
    Wq,
    bq,
    Wk,
    bk,
    ln_g,
    ln_b,
    labels,
    scenarios,
    sources,
    uids,
):
    global LAST_RESULTS
    nc = _get_nc()

    sim1 = np.ascontiguousarray(np.asarray(sim1, np.float32))
    sim2 = np.ascontiguousarray(np.asarray(sim2, np.float32))
    inp = np.ascontiguousarray(np.asarray(input_for_reconstruction, np.float32))
    Wq = np.ascontiguousarray(np.asarray(Wq, np.float32))
    Wk = np.ascontiguousarray(np.asarray(Wk, np.float32))
    bq = np.ascontiguousarray(np.asarray(bq, np.float32))
    bk = np.ascontiguousarray(np.asarray(bk, np.float32))
    ln_g = np.ascontiguousarray(np.asarray(ln_g, np.float32))
    ln_b = np.ascontiguousarray(np.asarray(ln_b, np.float32))
    uids_i = np.ascontiguousarray(np.asarray(uids, np.int32))

    in_maps = []
    for c in range(NCORES):
        sl = slice(c * BL, (c + 1) * BL)
        in_maps.append(
            {
                "xq": sim1[sl],
                "xk": sim2[sl],
                "inp": inp,
                "wq": Wq,
                "wk": Wk,
                "bq": bq,
                "bk": bk,
                "g": ln_g,
                "b": ln_b,
                "u_all": uids_i,
                "u_loc": np.ascontiguousarray(uids_i[sl]),
            }
        )

    res = run_bass_kernel_spmd(nc, in_maps, core_ids=list(range(NCORES)))
    LAST_RESULTS = res
    recon = np.concatenate([res.results[c]["recon"] for c in range(NCORES)], axis=0)
    sim_sm = np.concatenate([res.results[c]["sim_sm"] for c in range(NCORES)], axis=0)
    return (recon, sim_sm)


# revision 13
# speedup vs baseline: 143.4585x; 143.4585x over previous
"""Trainium2 Bass kernel for nn_CLIP_26869315404321 (contrastive loss_fn).

Computation (B=4096, D=2048, P=512):
    Q = LN(sim1 @ Wq + bq); S = LN(sim2 @ Wk + bk)
    sim = Q @ S.T - 1000*(eye + same_uid)   (eye is subsumed by same_uid)
    sim_sm = softmax(sim, axis=1)
    recon = sim_sm @ input_for_reconstruction
    returns (recon, sim_sm)

Sharding: batch dim B split across 8 cores (512 rows each). Each core
computes its own Q and S row-shard, the S shards are exchanged with an
on-device AllGather (packed fp16 hi/lo), and each core finishes its row
block of sim/softmax/recon. Host code slices inputs per core and
concatenates the row-shard outputs.

Precision: the projection and similarity matmuls use an fp16 hi/lo
3-matmul split (~1e-7 relative, needed because softmax amplifies absolute
logit error by sqrt(K)); the reconstruction matmul uses float32r.
"""

import os

import numpy as np

import bass_rust
import concourse.bass as bass
import concourse.mybir as mybir
import concourse.tile as tile_mod
from concourse.bass_utils import run_bass_kernel_spmd
from concourse.masks import make_identity
from concourse.tile import TileContext

# ---------------------------------------------------------------------------
# Compat: this container's walrus build rejects instructions carrying more
# than one sync-wait command. (1) Replace Tile's exit drain (which collects
# one wait per logical proc) with per-proc SP nops. (2) After tracing, hoist
# excess waits from any instruction onto same-engine NoOps placed just
# before it -- engine queues execute in order, so semantics are unchanged.
# ---------------------------------------------------------------------------


def _patched_drain(self, tick_clock, wait_clock):
    gc = tick_clock.global_clock
    vals = eval(repr(gc).replace("VectorClock", ""))
    n = len(vals)
    for proc in range(n):
        if vals[proc] > 0:
            single = [0] * n
            single[proc] = vals[proc]
            nop = self.nc.sync.nop(nofuse=True, hint="split_drain_wait")
            wait_clock.add_sem_waits(
                nop.ins, bass_rust.ScopedClock({None: bass_rust.VectorClock(single)})
            )
    self.nc.sync.drain()
    self.nc.all_engine_barrier()
    assert self.sems is not None
    popped = self.nc._tile_sem_poison_stack.pop()
    assert popped is self._sem_poison
    self.nc.clear_and_free_semaphores(list(self.sems.allocated().values()))
    self.nc.all_engine_barrier()


tile_mod.TileContext._drain_and_barrier = _patched_drain


def _split_multi_waits(nc, max_waits=1):
    n_split = 0
    for f in nc.m.functions:
        for blk in f.blocks:
            if not any(
                ins.sync_info is not None
                and ins.sync_info.on_wait
                and len(ins.sync_info.on_wait) > max_waits
                for ins in blk.instructions
            ):
                continue
            newlist = []
            for ins in blk.instructions:
                si = ins.sync_info
                waits = list(si.on_wait) if (si is not None and si.on_wait) else []
                if len(waits) > max_waits:
                    extra = waits[:-max_waits]
                    keep = waits[-max_waits:]
                    for i, w in enumerate(extra):
                        n_split += 1
                        newlist.append(
                            mybir.InstNoOp(
                                name=f"{ins.name}-hw{i}",
                                engine=ins.engine,
                                ins=[],
                                outs=[],
                                sync_info=mybir.SyncInfo(on_wait=[w], on_update=[]),
                            )
                        )
                    ins.sync_info = mybir.SyncInfo(
                        on_wait=keep, on_update=list(si.on_update or [])
                    )
                newlist.append(ins)
            blk.instructions[:] = newlist
    return n_split


# ---------------------------------------------------------------------------

B, D, P = 4096, 2048, 512
NCORES = 8
BL = B // NCORES  # 512 rows per core
RC = BL // 128  # 4 row chunks
KC = D // 128  # 16 contraction chunks for the projections
FC = P // 128  # 4 feature chunks
NS = B // 512  # 8 similarity column blocks
NJ = B // 128  # 32 contraction chunks for recon
LN_EPS = 1e-5
MASK_VAL = 1000.0

f32 = mybir.dt.float32
f32r = mybir.dt.float32r
f16 = mybir.dt.float16
i32 = mybir.dt.int32
ALU = mybir.AluOpType
ACTF = mybir.ActivationFunctionType
AX = mybir.AxisListType


def _build_module():
    from contextlib import ExitStack

    nc = bass.Bass(num_devices=NCORES)

    xq = nc.dram_tensor("xq", [BL, D], f32, kind="ExternalInput")
    xk = nc.dram_tensor("xk", [BL, D], f32, kind="ExternalInput")
    inp = nc.dram_tensor("inp", [B, D], f32, kind="ExternalInput")
    wq_d = nc.dram_tensor("wq", [D, P], f32, kind="ExternalInput")
    wk_d = nc.dram_tensor("wk", [D, P], f32, kind="ExternalInput")
    bq_d = nc.dram_tensor("bq", [P], f32, kind="ExternalInput")
    bk_d = nc.dram_tensor("bk", [P], f32, kind="ExternalInput")
    g_d = nc.dram_tensor("g", [P], f32, kind="ExternalInput")
    b_d = nc.dram_tensor("b", [P], f32, kind="ExternalInput")
    u_all_d = nc.dram_tensor("u_all", [B], i32, kind="ExternalInput")
    u_loc_d = nc.dram_tensor("u_loc", [BL], i32, kind="ExternalInput")

    recon_d = nc.dram_tensor("recon", [BL, D], f32, kind="ExternalOutput")
    sim_sm_d = nc.dram_tensor("sim_sm", [BL, B], f32, kind="ExternalOutput")

    # AllGather payload: fp16 hi rows [0:P], fp16 lo rows [P:2P], both
    # feature-major [feat, local_row].
    cc_in = nc.dram_tensor("cc_in", [2 * P, BL], f16, kind="Internal")
    cc_out = nc.dram_tensor(
        "cc_out", [NCORES * 2 * P, BL], f16, kind="Internal", addr_space="Shared"
    )

    with TileContext(nc) as tc, ExitStack() as ctx:
        const = ctx.enter_context(tc.tile_pool(name="const", bufs=1))
        small = ctx.enter_context(tc.tile_pool(name="small", bufs=8))

        ident = const.tile([128, 128], f32)
        make_identity(nc, ident[:])
        eps_t = const.tile([128, 1], f32)
        nc.gpsimd.memset(eps_t[:], LN_EPS)

        gpf = const.tile([128, FC], f32)
        nc.sync.dma_start(gpf[:], g_d.rearrange("(fc p) -> p fc", p=128))
        bpf = const.tile([128, FC], f32)
        nc.sync.dma_start(bpf[:], b_d.rearrange("(fc p) -> p fc", p=128))
        bq_b = const.tile([128, P], f32)
        nc.sync.dma_start(bq_b[:], bq_d[:].partition_broadcast(128))
        bk_b = const.tile([128, P], f32)
        nc.sync.dma_start(bk_b[:], bk_d[:].partition_broadcast(128))

        u_all_f = const.tile([128, B], f32)
        u_loc_f = const.tile([128, RC], f32)
        with tc.tile_pool(name="utmp", bufs=1) as utmp:
            u_all_i = utmp.tile([128, B], i32)
            nc.sync.dma_start(u_all_i[:], u_all_d[:].partition_broadcast(128))
            nc.vector.tensor_copy(u_all_f[:], u_all_i[:])
            u_loc_i = utmp.tile([128, RC], i32)
            nc.sync.dma_start(u_loc_i[:], u_loc_d.rearrange("(rc p) -> p rc", p=128))
            nc.vector.tensor_copy(u_loc_f[:], u_loc_i[:])

        # simT lives until the recon matmuls at the very end, so its pool is
        # opened first (pool frees must be LIFO).
        stt = ctx.enter_context(tc.tile_pool(name="stt", bufs=1))
        simT = stt.tile([128, NJ, BL], f32r, tag="simT")

        qsp = ctx.enter_context(tc.tile_pool(name="qsp", bufs=1))
        s_h = qsp.tile([128, FC, BL], f16, tag="sh")
        s_l = qsp.tile([128, FC, BL], f16, tag="sl")
        q_h = qsp.tile([128, FC, BL], f16, tag="qh")
        q_l = qsp.tile([128, FC, BL], f16, tag="ql")

        with (
            tc.tile_pool(name="wpool", bufs=2) as wpool,
            tc.tile_pool(name="xpool", bufs=1) as xpool,
            tc.tile_pool(name="xtp", bufs=3) as xtp,
            tc.tile_pool(name="prjp", bufs=2) as prjp,
            tc.tile_pool(name="pt_ps", bufs=3, space="PSUM") as pt_ps,
            tc.tile_pool(name="pp_ps", bufs=1, space="PSUM") as pp_ps,
        ):

            def proj_ln_t(x_d, w_d, bias_b, out_h, out_l, side):
                """out_h/out_l: [128, FC, BL] fp16 hi/lo of (LN(x@W+b)*g+b).T."""
                xnat = xpool.tile([128, RC, D], f32, tag="xnat")
                for rc in range(RC):
                    nc.sync.dma_start(
                        xnat[:, rc, :], x_d[rc * 128 : (rc + 1) * 128, :]
                    )
                psum_rc = [
                    pp_ps.tile([128, P], f32, tag=f"pp{rc}", name=f"pp{rc}_{side}")
                    for rc in range(RC)
                ]
                for kc in range(KC):
                    w_f = wpool.tile([128, P], f32, tag="w32")
                    nc.sync.dma_start(w_f[:], w_d[kc * 128 : (kc + 1) * 128, :])
                    w_h = wpool.tile([128, P], f16, tag="wh")
                    nc.scalar.copy(w_h[:], w_f[:])
                    w_l = wpool.tile([128, P], f16, tag="wl")
                    nc.vector.tensor_tensor(w_l[:], w_f[:], w_h[:], ALU.subtract)
                    xt_h = xtp.tile([128, RC, 128], f16, tag="xth")
                    xt_l = xtp.tile([128, RC, 128], f16, tag="xtl")
                    for rc in range(RC):
                        pst = pt_ps.tile([128, 128], f32, tag="pt")
                        nc.tensor.transpose(
                            pst[:], xnat[:, rc, kc * 128 : (kc + 1) * 128], ident[:]
                        )
                        nc.scalar.copy(xt_h[:, rc, :], pst[:])
                        nc.vector.tensor_tensor(
                            xt_l[:, rc, :], pst[:], xt_h[:, rc, :], ALU.subtract
                        )
                    for rc in range(RC):
                        for i, (lh, rh) in enumerate(
                            (
                                (xt_h[:, rc, :], w_h[:]),
                                (xt_h[:, rc, :], w_l[:]),
                                (xt_l[:, rc, :], w_h[:]),
                            )
                        ):
                            nc.tensor.matmul(
                                psum_rc[rc][:],
                                lh,
                                rh,
                                start=(kc == 0 and i == 0),
                                stop=(kc == KC - 1 and i == 2),
                            )

                for rc in range(RC):
                    prj = prjp.tile([128, P], f32, tag="prj")
                    nc.vector.tensor_tensor(
                        prj[:], psum_rc[rc][:], bias_b[:], ALU.add
                    )
                    st6 = small.tile([128, 6], f32, tag="st6")
                    nc.vector.bn_stats(st6[:], prj[:])
                    mv = small.tile([128, 2], f32, tag="mv")
                    nc.vector.bn_aggr(mv[:], st6[:])
                    std = small.tile([128, 1], f32, tag="s1")
                    nc.scalar.activation(std[:], mv[:, 1:2], ACTF.Sqrt, bias=eps_t[:])
                    rstd = small.tile([128, 1], f32, tag="s1")
                    nc.vector.reciprocal(rstd[:], std[:])
                    nmr = small.tile([128, 1], f32, tag="s1")
                    nc.vector.tensor_tensor(nmr[:], mv[:, 0:1], rstd[:], ALU.mult)
                    nc.vector.tensor_scalar(nmr[:], nmr[:], -1.0, None, ALU.mult)
                    tn = prjp.tile([128, P], f32, tag="tn")
                    nc.vector.tensor_scalar(
                        tn[:], prj[:], rstd[:], nmr[:], ALU.mult, ALU.add
                    )
                    # transpose back to feature-major, apply g/b, split hi/lo
                    for fc in range(FC):
                        pst = pt_ps.tile([128, 128], f32, tag="pt")
                        nc.tensor.transpose(
                            pst[:], tn[:, fc * 128 : (fc + 1) * 128], ident[:]
                        )
                        qf = prjp.tile([128, 128], f32, tag="qf")
                        nc.vector.tensor_scalar(
                            qf[:],
                            pst[:],
                            gpf[:, fc : fc + 1],
                            bpf[:, fc : fc + 1],
                            ALU.mult,
                            ALU.add,
                        )
                        hs = out_h[:, fc, rc * 128 : (rc + 1) * 128]
                        nc.scalar.copy(hs, qf[:])
                        nc.vector.tensor_tensor(
                            out_l[:, fc, rc * 128 : (rc + 1) * 128],
                            qf[:],
                            hs,
                            ALU.subtract,
                        )

            # ---------------- supports shard -> AllGather ----------------
            proj_ln_t(xk, wk_d, bk_b, s_h, s_l, "s")
            nc.sync.dma_start(
                cc_in[0:P, :].rearrange("(fc p) r -> p fc r", p=128), s_h[:]
            )
            nc.sync.dma_start(
                cc_in[P : 2 * P, :].rearrange("(fc p) r -> p fc r", p=128), s_l[:]
            )
            nc.gpsimd.collective_compute(
                "AllGather",
                ALU.bypass,
                ins=[cc_in[:]],
                outs=[cc_out[:]],
                replica_groups=[list(range(NCORES))],
            )

            # ------------- queries shard (overlaps the AllGather) -------------
            proj_ln_t(xq, wq_d, bq_b, q_h, q_l, "q")

        # ---------------- similarity + mask (ST streamed per column block) ----
        simp_es = ExitStack()
        simp = simp_es.enter_context(tc.tile_pool(name="simp", bufs=1))
        sim_sb = simp.tile([128, RC, B], f32, tag="sim")

        with (
            tc.tile_pool(name="stq", bufs=3) as stq,
            tc.tile_pool(name="mskp", bufs=2) as mskp,
            tc.tile_pool(name="sim_ps", bufs=3, space="PSUM") as sim_ps,
        ):
            for ns in range(NS):
                r = ns  # one gathered rank block per 512-column block
                base = r * 2 * P
                stb_h = stq.tile([128, FC, 512], f16, tag="sth")
                nc.sync.dma_start(
                    stb_h[:],
                    cc_out[base : base + P, :].rearrange("(fc p) j -> p fc j", p=128),
                )
                stb_l = stq.tile([128, FC, 512], f16, tag="stl")
                nc.sync.dma_start(
                    stb_l[:],
                    cc_out[base + P : base + 2 * P, :].rearrange(
                        "(fc p) j -> p fc j", p=128
                    ),
                )
                for mc in range(RC):
                    ps = sim_ps.tile([128, 512], f32, tag="ps")
                    i_mm = 0
                    n_mm = 3 * FC
                    for fc in range(FC):
                        for lh, rh in (
                            (q_h[:, fc, mc * 128 : (mc + 1) * 128], stb_h[:, fc, :]),
                            (q_h[:, fc, mc * 128 : (mc + 1) * 128], stb_l[:, fc, :]),
                            (q_l[:, fc, mc * 128 : (mc + 1) * 128], stb_h[:, fc, :]),
                        ):
                            nc.tensor.matmul(
                                ps[:],
                                lh,
                                rh,
                                start=(i_mm == 0),
                                stop=(i_mm == n_mm - 1),
                            )
                            i_mm += 1
                    msk = mskp.tile([128, 512], f32, tag="msk")
                    nc.vector.tensor_scalar(
                        msk[:],
                        u_all_f[:, ns * 512 : (ns + 1) * 512],
                        u_loc_f[:, mc : mc + 1],
                        -MASK_VAL,
                        ALU.is_equal,
                        ALU.mult,
                    )
                    nc.vector.tensor_tensor(
                        sim_sb[:, mc, ns * 512 : (ns + 1) * 512], ps[:], msk[:], ALU.add
                    )

        # ---------------- softmax (rows) + write sim_sm ----------------
        for mc in range(RC):
            row = sim_sb[:, mc, :]
            nmx = small.tile([128, 1], f32, tag="s1")
            nc.vector.tensor_reduce(nmx[:], row, axis=AX.X, op=ALU.max, negate=True)
            ssum = small.tile([128, 1], f32, tag="s1")
            nc.scalar.activation(
                row, row, ACTF.Exp, bias=nmx[:], scale=1.0, accum_out=ssum[:]
            )
            rs = small.tile([128, 1], f32, tag="s1")
            nc.vector.reciprocal(rs[:], ssum[:])
            nc.scalar.mul(row, row, rs[:])
            nc.scalar.dma_start(sim_sm_d[mc * 128 : (mc + 1) * 128, :], row)

        # ---------------- transpose sim_sm for recon ----------------
        with tc.tile_pool(name="pt2_ps", bufs=3, space="PSUM") as pt2_ps:
            for mc in range(RC):
                for j in range(NJ):
                    pst = pt2_ps.tile([128, 128], f32, tag="pt2")
                    nc.tensor.transpose(
                        pst[:], sim_sb[:, mc, j * 128 : (j + 1) * 128], ident[:]
                    )
                    dst = simT[:, j, mc * 128 : (mc + 1) * 128]
                    if (mc * NJ + j) % 2 == 0:
                        nc.vector.tensor_copy(dst, pst[:])
                    else:
                        nc.scalar.copy(dst, pst[:])
        simp_es.close()

        # ---------------- recon = sim_sm @ inp ----------------
        inpp = ctx.enter_context(tc.tile_pool(name="inpp", bufs=6))
        recp = ctx.enter_context(tc.tile_pool(name="recp", bufs=4))
        rec_ps = ctx.enter_context(tc.tile_pool(name="rec_ps", bufs=1, space="PSUM"))
        DG = 2  # halves of D; 8 psum banks = 4 row-chunks x 2 columns of 512
        for dg in range(DG):
            rps = [
                [
                    rec_ps.tile(
                        [128, 512], f32, tag=f"pr_{mc}_{dc}", name=f"pr_{mc}_{dc}"
                    )
                    for dc in range(2)
                ]
                for mc in range(RC)
            ]
            for j in range(NJ):
                it = inpp.tile([128, 1024], f32r, tag="it")
                nc.sync.dma_start(
                    it[:],
                    inp[j * 128 : (j + 1) * 128, dg * 1024 : (dg + 1) * 1024].bitcast(
                        f32r
                    ),
                )
                for mc in range(RC):
                    for dc in range(2):
                        nc.tensor.matmul(
                            rps[mc][dc][:],
                            simT[:, j, mc * 128 : (mc + 1) * 128],
                            it[:, dc * 512 : (dc + 1) * 512],
                            start=(j == 0),
                            stop=(j == NJ - 1),
                        )
            for mc in range(RC):
                for dc in range(2):
                    rec = recp.tile([128, 512], f32, tag="rec")
                    nc.vector.tensor_copy(rec[:], rps[mc][dc][:])
                    nc.scalar.dma_start(
                        recon_d[
                            mc * 128 : (mc + 1) * 128,
                            dg * 1024 + dc * 512 : dg * 1024 + (dc + 1) * 512,
                        ],
                        rec[:],
                    )

    _split_multi_waits(nc)
    return nc


_CACHE = {}
LAST_RESULTS = None
_last_in_maps = None


def _get_nc():
    if "nc" not in _CACHE:
        _CACHE["nc"] = _build_module()
    return _CACHE["nc"]


def _get_runner(nc):
    """Cached shard_map runner (mirrors bass2jax.run_bass_via_pjrt's
    multi-core path, but keeps the jitted executable across calls)."""
    if "runner" in _CACHE:
        return _CACHE["runner"]
    import jax
    from jax.sharding import Mesh, PartitionSpec

    try:
        from jax.experimental.shard_map import shard_map
    except ImportError:
        from jax.shard_map import shard_map  # newer jax

    from concourse import bass2jax

    bass2jax.install_neuronx_cc_hook()

    partition_name = nc.partition_id_tensor.name if nc.partition_id_tensor else None
    in_names = []
    out_names = []
    out_avals = []
    out_shapes = []
    for alloc in nc.m.functions[0].allocations:
        if not isinstance(alloc, mybir.MemoryLocationSet):
            continue
        name = alloc.memorylocations[0].name
        if alloc.kind == "ExternalInput":
            if name != partition_name:
                in_names.append(name)
        elif alloc.kind == "ExternalOutput":
            shape = tuple(alloc.tensor_shape)
            out_names.append(name)
            out_avals.append(jax.core.ShapedArray(shape, mybir.dt.np(alloc.dtype)))
            out_shapes.append((shape, mybir.dt.np(alloc.dtype)))
    n_params = len(in_names)
    all_in_names = in_names + out_names
    if partition_name is not None:
        all_in_names = all_in_names + [partition_name]

    def _body(*args):
        operands = list(args)
        if partition_name is not None:
            operands.append(bass2jax.partition_id_tensor())
        outs = bass2jax._bass_exec_p.bind(
            *operands,
            out_avals=tuple(out_avals),
            in_names=tuple(all_in_names),
            out_names=tuple(out_names),
            lowering_input_output_aliases=(),
            sim_require_finite=True,
            sim_require_nnan=True,
            nc=nc,
        )
        return tuple(outs)

    devices = jax.devices()[:NCORES]
    assert len(devices) == NCORES
    mesh = Mesh(np.asarray(devices), ("core",))
    n_outs = len(out_names)
    sharded = jax.jit(
        shard_map(
            _body,
            mesh=mesh,
            in_specs=(PartitionSpec("core"),) * (n_params + n_outs),
            out_specs=(PartitionSpec("core"),) * n_outs,
            check_rep=False,
        ),
        donate_argnums=tuple(range(n_params, n_params + n_outs)),
        keep_unused=True,
    )

    def run(in_maps):
        concat_in = [
            np.concatenate([m[name] for m in in_maps], axis=0) for name in in_names
        ]
        concat_zeros = [
            np.zeros((NCORES * s[0], *s[1:]), dt) for (s, dt) in out_shapes
        ]
        out_arrs = sharded(*concat_in, *concat_zeros)
        return [
            {
                name: np.asarray(out_arrs[i]).reshape(
                    NCORES, *out_shapes[i][0]
                )[c]
                for i, name in enumerate(out_names)
            }
            for c in range(NCORES)
        ]

    _CACHE["runner"] = run
    _CACHE["runner_parts"] = (in_names, out_names, out_shapes, mesh, _body, n_params)
    return run


def _build_nodon_runner(nc):
    """Timing-only variant: no donation, callable repeatedly on staged
    device arrays."""
    import jax
    from jax.sharding import Mesh, NamedSharding, PartitionSpec

    try:
        from jax.experimental.shard_map import shard_map
    except ImportError:
        from jax.shard_map import shard_map

    _get_runner(nc)
    in_names, out_names, out_shapes, mesh, _body, n_params = _CACHE["runner_parts"]
    n_outs = len(out_names)
    fn = jax.jit(
        shard_map(
            _body,
            mesh=mesh,
            in_specs=(PartitionSpec("core"),) * (n_params + n_outs),
            out_specs=(PartitionSpec("core"),) * n_outs,
            check_rep=False,
        ),
        keep_unused=True,
    )
    sh = NamedSharding(mesh, PartitionSpec("core"))

    def stage(in_maps):
        concat_in = [
            np.concatenate([m[name] for m in in_maps], axis=0) for name in in_names
        ]
        concat_zeros = [
            np.zeros((NCORES * s[0], *s[1:]), dt) for (s, dt) in out_shapes
        ]
        dev = [jax.device_put(a, sh) for a in concat_in + concat_zeros]
        jax.block_until_ready(dev)
        return in_names, dev

    out = {"fn": fn, "stage": stage}
    _CACHE["sharded_nodon"] = out
    return out


def kernel(
    sim1,
    sim2,
    input_for_reconstruction,
    Wq,
    bq,
    Wk,
    bk,
    ln_g,
    ln_b,
    labels,
    scenarios,
    sources,
    uids,
):
    global LAST_RESULTS
    nc = _get_nc()

    sim1 = np.ascontiguousarray(np.asarray(sim1, np.float32))
    sim2 = np.ascontiguousarray(np.asarray(sim2, np.float32))
    inp = np.ascontiguousarray(np.asarray(input_for_reconstruction, np.float32))
    Wq = np.ascontiguousarray(np.asarray(Wq, np.float32))
    Wk = np.ascontiguousarray(np.asarray(Wk, np.float32))
    bq = np.ascontiguousarray(np.asarray(bq, np.float32))
    bk = np.ascontiguousarray(np.asarray(bk, np.float32))
    ln_g = np.ascontiguousarray(np.asarray(ln_g, np.float32))
    ln_b = np.ascontiguousarray(np.asarray(ln_b, np.float32))
    uids_i = np.ascontiguousarray(np.asarray(uids, np.int32))

    in_maps = []
    for c in range(NCORES):
        sl = slice(c * BL, (c + 1) * BL)
        in_maps.append(
            {
                "xq": sim1[sl],
                "xk": sim2[sl],
                "inp": inp,
                "wq": Wq,
                "wk": Wk,
                "bq": bq,
                "bk": bk,
                "g": ln_g,
                "b": ln_b,
                "u_all": uids_i,
                "u_loc": np.ascontiguousarray(uids_i[sl]),
            }
        )

    global _last_in_maps
    _last_in_maps = in_maps
    results = _get_runner(nc)(in_maps)
    LAST_RESULTS = results
    recon = np.concatenate([results[c]["recon"] for c in range(NCORES)], axis=0)
    sim_sm = np.concatenate([results[c]["sim_sm"] for c in range(NCORES)], axis=0)
    return (recon, sim_sm)
